# revision 1
# baseline (speedup 1.0000x reference)
"""GTE contrastive loss kernel for 8 Trainium2 NeuronCores.

Math (reference): loss = -mean_i( cos(a_i,p_i)/T - log(partition_i) ),
partition_i = sum_j E_ap[i,j] + sum_j E_aa[i,j] + sum_j E_ap[j,i]
            + sum_j E_pp[j,i] - 2*exp(1/T),   E_xy = exp(cos/T).

Sharding: core k owns row block k (1024 rows).  Inputs are rotated by
-1024k rows per core so one SPMD program suffices: "my rows" are always
rows 0:1024 of the rotated input, and column block j means global block
(k+j) mod 8.  Symmetry of E_aa/E_pp lets each core compute only column
blocks 0..4: blocks 1..3 also emit column sums which cover the missing
row-sum pieces of blocks 5..7 on other cores; block 4 is computed by
both endpoint cores (rowsum only) so it is never double counted.
"""

import os
import sys

import numpy as np

for _p in ("/opt/trn_rl_repo", os.path.expanduser("/root/.axon_site/_ro/trn_rl_repo")):
    if os.path.isdir(_p) and _p not in sys.path:
        sys.path.insert(0, _p)

from concourse import bass, masks, tile  # noqa: E402
from concourse.bass_utils import run_bass_kernel_spmd  # noqa: E402

mybir = bass.mybir
F32 = mybir.dt.float32

N, D, NCORES = 8192, 64, 8
B = N // NCORES            # 1024 rows per core
MT = B // 128              # 8 row tiles of 128
TFULL = N // 128           # 64 transpose tiles
INV_T = 20.0

AP_BLOCKS = list(range(8))       # ap: all column blocks, all with colsum
SYM_BLOCKS = [0, 1, 2, 3, 4]     # aa/pp: cyclic half
SYM_COL_BLOCKS = [1, 2, 3]       # aa/pp blocks that also emit colsums


def _emit_load_normalize(nc, tc, ctx, dram_in, name):
    """DMA [N,D] -> [128,64,64] nat layout and row-normalize in place."""
    singles = ctx.enter_context(tc.tile_pool(name=f"{name}_sb", bufs=1))
    tmp = ctx.enter_context(tc.tile_pool(name=f"{name}_tmp", bufs=1))

    nat = singles.tile([128, TFULL, D], F32)
    sq = tmp.tile([128, TFULL, D], F32, tag="sqtmp")
    src = dram_in[:].rearrange("(t p) d -> p t d", p=128)
    for h in range(4):
        t0, t1 = h * (TFULL // 4), (h + 1) * (TFULL // 4)
        nc.sync.dma_start(out=nat[:, t0:t1, :], in_=src[:, t0:t1, :])
        nc.vector.tensor_mul(sq[:, t0:t1, :], nat[:, t0:t1, :], nat[:, t0:t1, :])
    ss = singles.tile([128, TFULL], F32)
    nc.vector.tensor_reduce(ss[:], sq[:], axis=mybir.AxisListType.X,
                            op=mybir.AluOpType.add)
    nrm = singles.tile([128, TFULL], F32)
    nc.scalar.activation(nrm[:], ss[:], mybir.ActivationFunctionType.Sqrt)
    inv = singles.tile([128, TFULL], F32)
    nc.vector.reciprocal(inv[:], nrm[:])
    # nat[p, t, d] *= inv[p, t]  (broadcast along d)
    inv_b = inv[:].broadcast_to([128, TFULL, D])
    nat2 = singles.tile([128, TFULL, D], F32, tag=f"{name}_nat2")
    nc.vector.tensor_mul(nat2[:], nat[:], inv_b)
    return singles, nat2


def _emit_transpose(nc, tc, singles, nat, ident, name):
    """PE-transpose normalized nat [128,64,64] -> xT [64, N]."""
    xT = singles.tile([64, N], F32, tag=f"{name}_xT")
    with tc.tile_pool(name=f"{name}_tr", bufs=2, space="PSUM") as trp:
        for q in range(TFULL // 4):
            tr = trp.tile([64, 512], F32, tag="tr")
            for s in range(4):
                t = q * 4 + s
                nc.tensor.transpose(tr[:, s * 128:(s + 1) * 128], nat[:, t, :],
                                    ident[:])
            nc.vector.tensor_copy(xT[:, q * 512:(q + 1) * 512], tr[:])
    return xT


def build_program():
    nc = bass.Bass()
    a_in = nc.declare_dram_parameter("a", [N, D], F32, isOutput=False)
    p_in = nc.declare_dram_parameter("p", [N, D], F32, isOutput=False)
    o_st_ap = nc.declare_dram_parameter("st_ap", [128, MT * 8], F32, isOutput=True)
    o_st_aa = nc.declare_dram_parameter("st_aa", [128, MT * 5], F32, isOutput=True)
    o_st_pp = nc.declare_dram_parameter("st_pp", [128, MT * 5], F32, isOutput=True)
    o_diag = nc.declare_dram_parameter("diag", [128, MT], F32, isOutput=True)
    o_cs_ap = nc.declare_dram_parameter("cs_ap", [1, 8 * B], F32, isOutput=True)
    o_cs_aa = nc.declare_dram_parameter("cs_aa", [1, 3 * B], F32, isOutput=True)
    o_cs_pp = nc.declare_dram_parameter("cs_pp", [1, 3 * B], F32, isOutput=True)

    with tile.TileContext(nc) as tc:
        import contextlib

        with contextlib.ExitStack() as ctx:
            a_sing, a_nat = _emit_load_normalize(nc, tc, ctx, a_in, "a")
            p_sing, p_nat = _emit_load_normalize(nc, tc, ctx, p_in, "p")

            res = ctx.enter_context(tc.tile_pool(name="results", bufs=1))
            st_ap = res.tile([128, MT * 8], F32)
            st_aa = res.tile([128, MT * 5], F32)
            st_pp = res.tile([128, MT * 5], F32)
            diag = res.tile([128, MT], F32)
            ones_t = res.tile([128, 128], F32)
            nc.vector.memset(ones_t[:], 1.0)

            # diagonal cos(a_i, p_i) for own rows (block 0 of rotated input)
            dtmp = res.tile([128, MT, D], F32)
            nc.vector.tensor_mul(dtmp[:], a_nat[:, 0:MT, :], p_nat[:, 0:MT, :])
            nc.vector.tensor_reduce(diag[:], dtmp[:], axis=mybir.AxisListType.X,
                                    op=mybir.AluOpType.add)

            ident0 = res.tile([128, 128], F32)
            masks.make_identity(nc, ident0[:])
            ident = res.tile([128, 128], F32)
            nc.vector.tensor_copy(ident[:], ident0[:])
            aT = _emit_transpose(nc, tc, a_sing, a_nat, ident, "a")
            pT = _emit_transpose(nc, tc, p_sing, p_nat, ident, "p")

            csp = ctx.enter_context(tc.tile_pool(name="csstage", bufs=2))
            mmp = ctx.enter_context(tc.tile_pool(name="mm", bufs=2, space="PSUM"))
            colp = ctx.enter_context(tc.tile_pool(name="col", bufs=2, space="PSUM"))
            ep = ctx.enter_context(tc.tile_pool(name="etile", bufs=3))

            jobs = []
            for g, j in enumerate(AP_BLOCKS):
                jobs.append((aT, pT, j, st_ap, g, 8, o_cs_ap, j))
            for g, j in enumerate(SYM_BLOCKS):
                cj = SYM_COL_BLOCKS.index(j) if j in SYM_COL_BLOCKS else None
                jobs.append((aT, aT, j, st_aa, g, 5, o_cs_aa, cj))
            for g, j in enumerate(SYM_BLOCKS):
                cj = SYM_COL_BLOCKS.index(j) if j in SYM_COL_BLOCKS else None
                jobs.append((pT, pT, j, st_pp, g, 5, o_cs_pp, cj))

            for xT, yT, j, st, g, ng, cs, cj in jobs:
                col_ps = None
                if cj is not None:
                    col_ps = colp.tile([128, B], F32, tag="col")
                for m in range(MT):
                    mm_ps = mmp.tile([128, B], F32, tag="mm")
                    for c in range(2):
                        nc.tensor.matmul(
                            mm_ps[:, c * 512:(c + 1) * 512],
                            xT[:, m * 128:(m + 1) * 128],
                            yT[:, j * B + c * 512: j * B + (c + 1) * 512],
                            start=True, stop=True,
                        )
                    e = ep.tile([128, B], F32, tag="e")
                    nc.scalar.activation(
                        e[:], mm_ps[:], mybir.ActivationFunctionType.Exp,
                        scale=INV_T,
                        accum_out=st[:, m * ng + g: m * ng + g + 1],
                    )
                    if col_ps is not None:
                        for c in range(2):
                            nc.tensor.matmul(
                                col_ps[:, c * 512:(c + 1) * 512],
                                ones_t[:],
                                e[:, c * 512:(c + 1) * 512],
                                start=(m == 0), stop=(m == MT - 1),
                                skip_group_check=True,
                            )
                if col_ps is not None:
                    cstage = csp.tile([1, B], F32, tag="cs")
                    nc.scalar.activation(cstage[:], col_ps[0:1, :],
                                         mybir.ActivationFunctionType.Copy)
                    nc.sync.dma_start(out=cs[0:1, cj * B:(cj + 1) * B],
                                      in_=cstage[:])

            nc.sync.dma_start(out=o_st_ap[:], in_=st_ap[:])
            nc.sync.dma_start(out=o_st_aa[:], in_=st_aa[:])
            nc.sync.dma_start(out=o_st_pp[:], in_=st_pp[:])
            nc.sync.dma_start(out=o_diag[:], in_=diag[:])
    return nc


def combine(core_outs):
    """core_outs: list (per core) of dicts with the 7 output arrays."""
    self_term = np.exp(np.float32(INV_T))
    rs_ap = np.empty(N, np.float32)
    rs_aa = np.empty(N, np.float32)
    rs_pp = np.empty(N, np.float32)
    diag = np.empty(N, np.float32)
    cs_ap_tot = np.zeros(N, np.float64)
    aa_contrib = np.zeros(N, np.float64)
    pp_contrib = np.zeros(N, np.float64)

    for k, o in enumerate(core_outs):
        sl = slice(k * B, (k + 1) * B)
        # st[p, m*ng+g] -> local row m*128+p; sum over g
        rs_ap[sl] = o["st_ap"].reshape(128, MT, 8).sum(-1).T.reshape(B)
        rs_aa[sl] = o["st_aa"].reshape(128, MT, 5).sum(-1).T.reshape(B)
        rs_pp[sl] = o["st_pp"].reshape(128, MT, 5).sum(-1).T.reshape(B)
        diag[sl] = o["diag"].T.reshape(B)

        cs_ap_tot += np.roll(o["cs_ap"].reshape(N).astype(np.float64), k * B)
        for row, j in enumerate(SYM_COL_BLOCKS):
            v = np.zeros(N, np.float64)
            v[j * B:(j + 1) * B] = o["cs_aa"].reshape(3, B)[row]
            aa_contrib += np.roll(v, k * B)
            v = np.zeros(N, np.float64)
            v[j * B:(j + 1) * B] = o["cs_pp"].reshape(3, B)[row]
            pp_contrib += np.roll(v, k * B)

    partition = (rs_ap.astype(np.float64) + cs_ap_tot
                 + rs_aa.astype(np.float64) + aa_contrib
                 + rs_pp.astype(np.float64) + pp_contrib
                 - 2.0 * float(self_term))
    pos_logit = INV_T * diag.astype(np.float64)
    loss = -(pos_logit - np.log(partition)).mean()
    return np.float32(loss)


def _split_waits(nc):
    """Walrus codegen allows ~1 sync wait per instruction; hoist extra
    waits onto same-engine NoOps inserted just before the instruction."""
    for fn in nc.m.functions:
        for blk in fn.blocks:
            new = []
            for inst in blk.instructions:
                si = getattr(inst, "sync_info", None)
                keep = 1
                if si is not None and si.on_wait and len(si.on_wait) > keep:
                    waits = list(si.on_wait)
                    for i, w in enumerate(waits[:-keep]):
                        nop = mybir.InstNoOp(name=f"{inst.name}-sw{i}")
                        nop.engine = inst.engine
                        nop.sync_info = mybir.SyncInfo(on_wait=[w], on_update=[])
                        new.append(nop)
                    inst.sync_info = mybir.SyncInfo(
                        on_wait=list(waits[-keep:]),
                        on_update=list(si.on_update))
                new.append(inst)
            blk.instructions = new


_NC_CACHE = None


def _get_program():
    global _NC_CACHE
    if _NC_CACHE is None:
        _NC_CACHE = build_program()
        _split_waits(_NC_CACHE)
    return _NC_CACHE


def run(anchor_embeddings, positive_embeddings, trace=False, **trace_kwargs):
    a = np.ascontiguousarray(anchor_embeddings, dtype=np.float32)
    p = np.ascontiguousarray(positive_embeddings, dtype=np.float32)
    in_maps = [
        {"a": np.roll(a, -k * B, axis=0), "p": np.roll(p, -k * B, axis=0)}
        for k in range(NCORES)
    ]
    nc = _get_program()
    res = run_bass_kernel_spmd(nc, in_maps, list(range(NCORES)), trace=trace,
                               **trace_kwargs)
    return combine(res.results), res


def kernel(anchor_embeddings, positive_embeddings):
    loss, _ = run(anchor_embeddings, positive_embeddings)
    return loss



# revision 13
# speedup vs baseline: 1.8072x; 1.8072x over previous
"""GTE contrastive loss kernel for 8 Trainium2 NeuronCores.

Math (reference): loss = -mean_i( cos(a_i,p_i)/T - log(partition_i) ),
partition_i = sum_j E_ap[i,j] + sum_j E_aa[i,j] + sum_j E_ap[j,i]
            + sum_j E_pp[j,i] - 2*exp(1/T),   E_xy = exp(cos/T).

Sharding: core k owns row block k (1024 rows).  Inputs are rotated by
-1024k rows per core so one SPMD program suffices: "my rows" are always
rows 0:1024 of the rotated input, and column block j means global block
(k+j) mod 8.  Symmetry of E_aa/E_pp lets each core compute only column
blocks 0..4: blocks 1..3 also emit column sums which cover the missing
row-sum pieces of blocks 5..7 on other cores; block 4 is computed by
both endpoint cores (rowsum only) so it is never double counted.

v2 performance structure:
 - all matmuls run in float32r (bitcast of the f32 data; 1 PE cycle/row
   vs 4 for plain f32 at moving size >= 512)
 - 18 column blocks per row-tile are processed as 9 pairs; one Exp
   activation per pair covers [128, 2048] with a single accum_out
   (row-sum slots are summed on the host, so mixing blocks is fine)
 - exp output is bf16; per-block column sums accumulate on the DVE
   (bf16 2x mode) into SBUF, with a final ones-matmul partition reduce
 - load/normalize temporaries live in a pool that closes before the
   main loop so SBUF fits
"""

import os
import sys

import numpy as np

for _p in ("/opt/trn_rl_repo", os.path.expanduser("/root/.axon_site/_ro/trn_rl_repo")):
    if os.path.isdir(_p) and _p not in sys.path:
        sys.path.insert(0, _p)

from concourse import bass, masks, tile  # noqa: E402
from concourse.bass_utils import run_bass_kernel_spmd  # noqa: E402

mybir = bass.mybir
F32 = mybir.dt.float32
F32R = mybir.dt.float32r
BF16 = mybir.dt.bfloat16

N, D, NCORES = 8192, 64, 8
B = N // NCORES            # 1024 rows per core
MT = B // 128              # 8 row tiles of 128
TFULL = N // 128           # 64 transpose tiles
INV_T = 20.0

# blocks computed per core (see module docstring)
AP_BLOCKS = list(range(8))       # ap: all column blocks, all with colsum
SYM_BLOCKS = [0, 1, 2, 3, 4]     # aa/pp: cyclic half
SYM_COL_BLOCKS = [1, 2, 3]       # aa/pp blocks that also emit colsums

# activation pair schedule: 9 pairs of (matrix, block); one [128, 2048]
# Exp per pair with a single shared accum_out slot.
PAIRS = [
    [("ap", 0), ("ap", 1)],
    [("ap", 2), ("ap", 3)],
    [("ap", 4), ("ap", 5)],
    [("ap", 6), ("ap", 7)],
    [("aa", 0), ("aa", 1)],
    [("aa", 2), ("aa", 3)],
    [("pp", 0), ("pp", 1)],
    [("pp", 2), ("pp", 3)],
    [("aa", 4), ("pp", 4)],
]
NSLOT = len(PAIRS)

# column-sum accumulator index per (matrix, block); 14 total
_COLACC = {}
for _j in AP_BLOCKS:
    _COLACC[("ap", _j)] = _j
for _i, _j in enumerate(SYM_COL_BLOCKS):
    _COLACC[("aa", _j)] = 8 + _i
    _COLACC[("pp", _j)] = 11 + _i
NCS = len(_COLACC)


def _emit_load_normalize(nc, tc, ldp, tmp, dram_in, name):
    """DMA [N,D] -> [128,64,64] nat layout and row-normalize."""
    nat = ldp.tile([128, TFULL, D], F32, tag=f"{name}_nat")
    sq = tmp.tile([128, TFULL, D], F32, tag="sqtmp")
    src = dram_in[:].rearrange("(t p) d -> p t d", p=128)
    for h in range(4):
        t0, t1 = h * (TFULL // 4), (h + 1) * (TFULL // 4)
        nc.sync.dma_start(out=nat[:, t0:t1, :], in_=src[:, t0:t1, :])
        nc.vector.tensor_mul(sq[:, t0:t1, :], nat[:, t0:t1, :], nat[:, t0:t1, :])
    ss = tmp.tile([128, TFULL], F32, tag="sstmp")
    nc.vector.tensor_reduce(ss[:], sq[:], axis=mybir.AxisListType.X,
                            op=mybir.AluOpType.add)
    nrm = tmp.tile([128, TFULL], F32, tag="nrmtmp")
    nc.scalar.activation(nrm[:], ss[:], mybir.ActivationFunctionType.Sqrt)
    inv = tmp.tile([128, TFULL], F32, tag="invtmp")
    nc.vector.reciprocal(inv[:], nrm[:])
    # nat[p, t, d] *= inv[p, t]  (broadcast along d)
    inv_b = inv[:].broadcast_to([128, TFULL, D])
    nat2 = ldp.tile([128, TFULL, D], F32, tag=f"{name}_nat2")
    nc.vector.tensor_mul(nat2[:], nat[:], inv_b)
    return nat2


def _emit_transpose(nc, tc, singles, nat, ident, name, copy_engines):
    """PE-transpose normalized nat [128,64,64] -> xT [64, N] (f32r) plus
    an exact f32 copy of the first block column, xT32 [64, B].

    The f32r rounding (~1e-4) is fine off-diagonal, but the self-diagonal
    blocks need cos(i,i)=1 to ~1e-6 so the host's analytic exp(1/T)
    subtraction cancels; those matmuls read the f32 copy.

    PSUM staging in [64, 2048] chunks; the PSUM->SBUF copies round-robin
    over the given engines so no single engine eats the whole cost.
    """
    QW = 2048                      # copy width (4 PSUM banks)
    TQ = QW // 128                 # transposes per chunk
    # float32r so the PSUM->SBUF copy rounds once; the fp32r matmuls then
    # consume it directly (verifier requires producer-side rounding)
    xT = singles.tile([64, N], F32R, tag=f"{name}_xT")
    xT32 = singles.tile([64, B], F32, tag=f"{name}_xT32")
    with tc.tile_pool(name=f"{name}_tr", bufs=2, space="PSUM") as trp:
        for q in range(N // QW):
            tr = trp.tile([64, QW], F32, tag="tr")
            for s in range(TQ):
                t = q * TQ + s
                nc.tensor.transpose(tr[:, s * 128:(s + 1) * 128], nat[:, t, :],
                                    ident[:])
            eng = copy_engines[q % len(copy_engines)]
            if hasattr(eng, "tensor_copy"):
                eng.tensor_copy(xT[:, q * QW:(q + 1) * QW], tr[:])
            else:
                eng.copy(xT[:, q * QW:(q + 1) * QW], tr[:])
            if q == 0:
                eng2 = copy_engines[(q + 1) % len(copy_engines)]
                if hasattr(eng2, "tensor_copy"):
                    eng2.tensor_copy(xT32[:], tr[:, 0:B])
                else:
                    eng2.copy(xT32[:], tr[:, 0:B])
    return xT, xT32


def build_program():
    nc = bass.Bass()
    a_in = nc.declare_dram_parameter("a", [N, D], F32, isOutput=False)
    p_in = nc.declare_dram_parameter("p", [N, D], F32, isOutput=False)
    o_st = nc.declare_dram_parameter("st", [128, MT * NSLOT], F32, isOutput=True)
    o_diag = nc.declare_dram_parameter("diag", [128, MT], F32, isOutput=True)
    o_cs_ap = nc.declare_dram_parameter("cs_ap", [1, 8 * B], F32, isOutput=True)
    o_cs_aa = nc.declare_dram_parameter("cs_aa", [1, 3 * B], F32, isOutput=True)
    o_cs_pp = nc.declare_dram_parameter("cs_pp", [1, 3 * B], F32, isOutput=True)

    with tile.TileContext(nc) as tc:
        import contextlib

        with contextlib.ExitStack() as ctx:
            res = ctx.enter_context(tc.tile_pool(name="results", bufs=1))
            st = res.tile([128, MT * NSLOT], F32)
            diag = res.tile([128, MT], F32)
            ones_bf = res.tile([128, 128], BF16)
            nc.vector.memset(ones_bf[:], 1.0)
            ident0 = res.tile([128, 128], F32)
            masks.make_identity(nc, ident0[:])
            ident = res.tile([128, 128], F32)
            nc.vector.tensor_copy(ident[:], ident0[:])

            xts = ctx.enter_context(tc.tile_pool(name="xts", bufs=1))
            colacc = res.tile([128, NCS, B], BF16)

            with tc.tile_pool(name="ldtmp", bufs=1) as ldp, \
                    tc.tile_pool(name="ldtmp2", bufs=1) as tmp:
                a_nat = _emit_load_normalize(nc, tc, ldp, tmp, a_in, "a")
                p_nat = _emit_load_normalize(nc, tc, ldp, tmp, p_in, "p")

                # diagonal cos(a_i, p_i) for own rows (block 0 of rotation)
                dtmp = tmp.tile([128, MT, D], F32, tag="dtmp")
                nc.vector.tensor_mul(dtmp[:], a_nat[:, 0:MT, :], p_nat[:, 0:MT, :])
                nc.vector.tensor_reduce(diag[:], dtmp[:],
                                        axis=mybir.AxisListType.X,
                                        op=mybir.AluOpType.add)

                # NB: GPSIMD (Pool) cannot access PSUM; only DVE/ACT copy.
                aT, aT32 = _emit_transpose(nc, tc, xts, a_nat, ident, "a",
                                           [nc.vector, nc.scalar])
                pT, pT32 = _emit_transpose(nc, tc, xts, p_nat, ident, "p",
                                           [nc.scalar, nc.vector])
            # ldtmp pools closed: nat/nat2/sq space is free again

            srcs = {"ap": (aT, pT), "aa": (aT, aT), "pp": (pT, pT)}

            with tc.tile_pool(name="mm", bufs=2, space="PSUM") as mmp, \
                    tc.tile_pool(name="etile", bufs=3) as ep:
                for m in range(MT):
                    for si, pair in enumerate(PAIRS):
                        mm_ps = mmp.tile([128, 2 * B], F32, tag="mm")
                        for h, (mat, j) in enumerate(pair):
                            if j == 0 and mat in ("aa", "pp"):
                                # self-diagonal block: full fp32 for exact
                                # exp(1/T) cancellation on the host
                                x32 = aT32 if mat == "aa" else pT32
                                xs = ys = x32
                                lhs = x32[:, m * 128:(m + 1) * 128]
                                ybase = 0
                            else:
                                xs, ys = srcs[mat]
                                lhs = xs[:, m * 128:(m + 1) * 128]
                                ybase = j * B
                            for c in range(2):
                                o0 = h * B + c * 512
                                nc.tensor.matmul(
                                    mm_ps[:, o0:o0 + 512],
                                    lhs,
                                    ys[:, ybase + c * 512:
                                       ybase + (c + 1) * 512],
                                    start=True, stop=True,
                                )
                        e = ep.tile([128, 2 * B], BF16, tag="e")
                        slot = m * NSLOT + si
                        nc.scalar.activation(
                            e[:], mm_ps[:], mybir.ActivationFunctionType.Exp,
                            scale=INV_T,
                            accum_out=st[:, slot:slot + 1],
                        )
                        for h, (mat, j) in enumerate(pair):
                            ci = _COLACC.get((mat, j))
                            if ci is None:
                                continue
                            eh = e[:, h * B:(h + 1) * B]
                            if m == 0:
                                nc.vector.tensor_copy(colacc[:, ci, :], eh)
                            else:
                                nc.vector.tensor_add(colacc[:, ci, :],
                                                     colacc[:, ci, :], eh)

            # final partition reduce of the column sums: ones^T @ colacc
            with tc.tile_pool(name="colred", bufs=2, space="PSUM") as colp, \
                    tc.tile_pool(name="csstage", bufs=2) as csp:
                for (mat, j), ci in _COLACC.items():
                    cps = colp.tile([128, B], F32, tag="cps")
                    for c in range(2):
                        nc.tensor.matmul(
                            cps[:, c * 512:(c + 1) * 512],
                            ones_bf[:],
                            colacc[:, ci, c * 512:(c + 1) * 512],
                            start=True, stop=True,
                        )
                    cstage = csp.tile([1, B], F32, tag="cs")
                    nc.vector.tensor_copy(cstage[:], cps[0:1, :])
                    dst = {"ap": o_cs_ap, "aa": o_cs_aa, "pp": o_cs_pp}[mat]
                    cj = j if mat == "ap" else SYM_COL_BLOCKS.index(j)
                    nc.sync.dma_start(out=dst[0:1, cj * B:(cj + 1) * B],
                                      in_=cstage[:])

            nc.sync.dma_start(out=o_st[:], in_=st[:])
            nc.sync.dma_start(out=o_diag[:], in_=diag[:])
    return nc


def combine(core_outs):
    """core_outs: list (per core) of dicts with the 5 output arrays."""
    self_term = np.exp(np.float32(INV_T))
    rs = np.empty(N, np.float64)
    diag = np.empty(N, np.float32)
    cs_ap_tot = np.zeros(N, np.float64)
    aa_contrib = np.zeros(N, np.float64)
    pp_contrib = np.zeros(N, np.float64)

    for k, o in enumerate(core_outs):
        sl = slice(k * B, (k + 1) * B)
        # st[p, m*NSLOT+s] -> local row m*128+p; sum over s
        rs[sl] = o["st"].reshape(128, MT, NSLOT).astype(np.float64).sum(-1) \
                        .T.reshape(B)
        diag[sl] = o["diag"].T.reshape(B)

        cs_ap_tot += np.roll(o["cs_ap"].reshape(N).astype(np.float64), k * B)
        for row, j in enumerate(SYM_COL_BLOCKS):
            v = np.zeros(N, np.float64)
            v[j * B:(j + 1) * B] = o["cs_aa"].reshape(3, B)[row]
            aa_contrib += np.roll(v, k * B)
            v = np.zeros(N, np.float64)
            v[j * B:(j + 1) * B] = o["cs_pp"].reshape(3, B)[row]
            pp_contrib += np.roll(v, k * B)

    partition = (rs + cs_ap_tot + aa_contrib + pp_contrib
                 - 2.0 * float(self_term))
    pos_logit = INV_T * diag.astype(np.float64)
    loss = -(pos_logit - np.log(partition)).mean()
    return np.float32(loss)


def _split_waits(nc):
    """Walrus codegen allows ~1 sync wait per instruction; hoist extra
    waits onto same-engine NoOps inserted just before the instruction."""
    for fn in nc.m.functions:
        for blk in fn.blocks:
            new = []
            for inst in blk.instructions:
                si = getattr(inst, "sync_info", None)
                keep = 1
                if si is not None and si.on_wait and len(si.on_wait) > keep:
                    waits = list(si.on_wait)
                    for i, w in enumerate(waits[:-keep]):
                        nop = mybir.InstNoOp(name=f"{inst.name}-sw{i}")
                        nop.engine = inst.engine
                        nop.sync_info = mybir.SyncInfo(on_wait=[w], on_update=[])
                        new.append(nop)
                    inst.sync_info = mybir.SyncInfo(
                        on_wait=list(waits[-keep:]),
                        on_update=list(si.on_update))
                new.append(inst)
            blk.instructions = new


_NC_CACHE = None


def _get_program():
    global _NC_CACHE
    if _NC_CACHE is None:
        _NC_CACHE = build_program()
        _split_waits(_NC_CACHE)
    return _NC_CACHE


def run(anchor_embeddings, positive_embeddings, trace=False, **trace_kwargs):
    a = np.ascontiguousarray(anchor_embeddings, dtype=np.float32)
    p = np.ascontiguousarray(positive_embeddings, dtype=np.float32)
    in_maps = [
        {"a": np.roll(a, -k * B, axis=0), "p": np.roll(p, -k * B, axis=0)}
        for k in range(NCORES)
    ]
    nc = _get_program()
    res = run_bass_kernel_spmd(nc, in_maps, list(range(NCORES)), trace=trace,
                               **trace_kwargs)
    return combine(res.results), res


def kernel(anchor_embeddings, positive_embeddings):
    loss, _ = run(anchor_embeddings, positive_embeddings)
    return loss
